# revision 28
# baseline (speedup 1.0000x reference)
"""GAT (2-layer, PyG GATConv semantics) on 8 Trainium2 NeuronCores.

Strategy (dst-sharded edge parallelism, sharded prologue). The wall-clock
cost of a call in this environment is dominated by host->device transfer
over the axon tunnel (~73 MB/s), per-call jit re-lowering (scales with
BIR size), and dispatch latency -- actual device compute is only a few
ms. The design minimizes bytes moved and program size:

  - Append self-loops, sort edges by dst. Core k owns dst nodes
    [k*2560, (k+1)*2560) (N padded 20000 -> 20480), as 20 blocks of 128.
  - Prologue is node-sharded: core k computes h = x_k@W1 (+ per-node
    attention logits alpha_src/alpha_dst fused as extra matmul columns)
    only for its own 2560 nodes, then one AllGather builds the full packed
    row table hpk [20480, 320] on every core. x is fed pre-transposed and
    in fp8-e4m3 (output rel err ~1e-3, tolerance 2e-2).
  - Edge processing gathers hpk[src] rows with dma_gather (1280B rows).
    alpha_dst[dst] is gathered from the LOCAL table hpin (dst nodes are
    always core-own), via a narrow 256B-row view (elem_step); the local
    dst indices (b*128 + dst_local) are derived on device from the u8
    wrapped dstl table + a per-block iota base. One zeroed pad block
    absorbs sentinel indices of padding slots.
  - Per-tile one-hot matrices (from the same dstl table, regrouped via
    SBUF DMAs) drive PE matmuls that scatter-add softmax-weighted
    messages + denominators into PSUM.
  - Softmax without max-subtraction (logits are O(1); identical result).
  - One AllGather exchanges packed layer-2 features (g = ELU(y1)@W2 plus
    logits) between the layers; layer-2 dst logits again come from the
    local gpin table. Final log_softmax per dst row, written in bf16
    (rel err of bf16 is ~4e-3 for any magnitude) and cast to f32 on host.
  - Weights are uploaded sharded (1/8 per core) and reassembled with a
    small AllGather; constants (identity, iota) are generated on-device;
    all 20-block phases are For_i hardware loops, keeping the BIR at
    ~0.6MB so per-call re-lowering is cheap; a persistent XLA compile
    cache skips the NEFF pipeline on repeat compiles.
"""

import math

import numpy as np

# ---- problem constants (hardcoded per contract) ----
N = 20000
F = 128
HEADS = 8
CH = 32
HC = HEADS * CH  # 256
CLS = 40
E0 = 640000
NEG = 0.2
CORES = 8
BLK = 128
BPC = 20  # blocks per core
NPC = BLK * BPC  # 2560 nodes per core
NPAD = NPC * CORES  # 20480
HP_W = 320  # packed h row: [h(256) | a_src(8) | a_dst(8) | pad] -> 1280B
GP_W = 64  # packed g row: [g(40) | as2(1) | ad2(1) | pad] -> 256B
CN = 7  # tiles per gather chunk

# flat weights blob: [W1 | W2 | W2T | Ablk | a2sd | b1 | b2], sharded
# across cores and AllGather-reassembled on device (saves 7/8 of the
# replicated-weight upload)
_W_OFF = {}
_off = 0
for _name, _sz in [("W1", F * HC), ("W2", HC * CLS), ("W2T", CLS * HC),
                   ("Ablk", HC * 16), ("a2sd", CLS * 2), ("b1", HC),
                   ("b2", CLS)]:
    _W_OFF[_name] = _off
    _off += _sz
WSEG = -(-_off // (8 * 16)) * 16  # per-core segment, padded
WSZ = WSEG * 8

_cache = {}


def _wrap_idx16(idx):
    """Compact dma_gather index layout: [16, len//16] int16, idx i at
    [i%16, i//16]. Replication to 128 partitions happens on-device."""
    assert len(idx) % 16 == 0
    return np.ascontiguousarray(idx.astype(np.int16).reshape(-1, 16).T)


def _prep_edges(edge_index):
    src = np.asarray(edge_index[0], dtype=np.int64)
    dst = np.asarray(edge_index[1], dtype=np.int64)
    loops = np.arange(N, dtype=np.int64)
    src = np.concatenate([src, loops])
    dst = np.concatenate([dst, loops])
    order = np.argsort(dst, kind="stable")
    ssrc = src[order]
    sdst = dst[order]

    nblocks = NPAD // BLK  # 160
    counts = np.bincount(sdst // BLK, minlength=nblocks)
    starts = np.concatenate([[0], np.cumsum(counts)])
    tmax = CN * int(math.ceil(counts.max() / 128 / CN))
    nchunks = tmax // CN

    per_core = []
    for k in range(CORES):
        gsrc_cols = []
        gwdl_cols = []
        for b in range(BPC):
            g = k * BPC + b
            e0, e1 = starts[g], starts[g + 1]
            npadded = tmax * 128
            s = np.zeros(npadded, dtype=np.int64)
            dl = np.full(npadded, 128, dtype=np.uint8)  # 128 = dead sentinel
            s[: e1 - e0] = ssrc[e0:e1]
            dl[: e1 - e0] = (sdst[e0:e1] - g * BLK).astype(np.uint8)
            for c in range(nchunks):
                sl = slice(c * CN * 128, (c + 1) * CN * 128)
                gsrc_cols.append(_wrap_idx16(s[sl]))
                gwdl_cols.append(
                    np.ascontiguousarray(dl[sl].reshape(-1, 16).T))
        gsrc = np.concatenate(gsrc_cols, axis=1)  # [16, BPC*tmax*8] i16
        gwdl = np.concatenate(gwdl_cols, axis=1)  # [16, BPC*tmax*8] u8
        per_core.append({"gsrc": gsrc, "gwdl": gwdl})
    return tmax, per_core


def _build_nc(tmax):
    import concourse.bacc as bacc
    import concourse.mybir as mybir
    import concourse.tile as tile
    from concourse.bass import ts

    fp32 = mybir.dt.float32
    bf16 = mybir.dt.bfloat16
    i16 = mybir.dt.int16
    u8 = mybir.dt.uint8
    f8e4 = mybir.dt.float8e4
    ALU = mybir.AluOpType
    ACT = mybir.ActivationFunctionType

    nchunks = tmax // CN
    IW = BPC * tmax * 8  # index-table width (int16 cols)

    nc = bacc.Bacc("TRN2", target_bir_lowering=False, num_swdge_queues=4)

    # ---- I/O ----
    xT_t = nc.dram_tensor("xT", [F, NPC], f8e4, kind="ExternalInput")
    wseg_t = nc.dram_tensor("wseg", [WSEG], fp32, kind="ExternalInput")
    gsrc_t = nc.dram_tensor("gsrc", [16, IW], i16, kind="ExternalInput")
    gwdl_t = nc.dram_tensor("gwdl", [16, IW], u8, kind="ExternalInput")
    out_t = nc.dram_tensor("out", [NPC, CLS], bf16, kind="ExternalOutput")

    wsin_t = nc.dram_tensor("wsin", [WSEG], fp32)
    wpk_t = nc.dram_tensor("wpk", [WSZ], fp32, addr_space="Shared")
    hpin_t = nc.dram_tensor("hpin", [NPC + 128, HP_W], fp32)
    h2_t = nc.dram_tensor("h2", [NPC, HC], fp32)
    hpk_t = nc.dram_tensor("hpk", [NPAD, HP_W], fp32, addr_space="Shared")
    gpin_t = nc.dram_tensor("gpin", [NPC + 128, GP_W], fp32)
    gpk_t = nc.dram_tensor("gpk", [NPAD, GP_W], fp32, addr_space="Shared")

    OW = _W_OFF

    with tile.TileContext(nc) as tc:
        with (
            tc.tile_pool(name="const", bufs=1) as cp,
            tc.tile_pool(name="sb", bufs=2) as sb,
            tc.tile_pool(name="oh", bufs=2) as ohp,
        ):
            # ---- on-device constants ----
            iotaF = cp.tile([128, 128], fp32)
            nc.gpsimd.iota(iotaF[:], pattern=[[1, 128]], base=0,
                           channel_multiplier=0,
                           allow_small_or_imprecise_dtypes=True)
            iotaP = cp.tile([128, 1], fp32)
            nc.gpsimd.iota(iotaP[:], pattern=[[0, 1]], base=0,
                           channel_multiplier=1,
                           allow_small_or_imprecise_dtypes=True)
            ident = cp.tile([128, 128], fp32)
            nc.vector.tensor_tensor(out=ident[:],
                                    in0=iotaP[:].to_broadcast([128, 128]),
                                    in1=iotaF[:], op=ALU.is_equal)

            # ---- weights: AllGather the sharded blob, then unpack ----
            # (collectives can't read IO tensors: stage input seg via SBUF)
            wstage = cp.tile([16, WSEG // 16], fp32)
            nc.sync.dma_start(
                wstage[:], wseg_t[:].rearrange("(p a) -> p a", p=16))
            nc.sync.dma_start(
                wsin_t[:].rearrange("(p a) -> p a", p=16), wstage[:])
            nc.gpsimd.collective_compute(
                "AllGather",
                mybir.AluOpType.bypass,
                replica_groups=[list(range(CORES))],
                ins=[wsin_t[:]],
                outs=[wpk_t[:]],
            )
            w1 = cp.tile([128, HC], fp32)
            nc.sync.dma_start(
                w1[:], wpk_t[OW["W1"] : OW["W1"] + F * HC]
                .rearrange("(p a) -> p a", p=128))
            ablk = cp.tile([128, 2, 16], fp32)
            nc.sync.dma_start(
                ablk[:], wpk_t[OW["Ablk"] : OW["Ablk"] + HC * 16]
                .rearrange("(h p a) -> p h a", p=128, a=16))
            b1w = cp.tile([1, HC], fp32)
            nc.sync.dma_start(
                b1w[:], wpk_t[OW["b1"] : OW["b1"] + HC]
                .rearrange("(o a) -> o a", o=1))
            w2h = cp.tile([128, 2, CLS], fp32)
            nc.sync.dma_start(
                w2h[:], wpk_t[OW["W2"] : OW["W2"] + HC * CLS]
                .rearrange("(h p a) -> p h a", p=128, a=CLS))
            w2T = cp.tile([CLS, HC], fp32)
            nc.sync.dma_start(
                w2T[:], wpk_t[OW["W2T"] : OW["W2T"] + CLS * HC]
                .rearrange("(p a) -> p a", p=CLS))
            a2 = cp.tile([CLS, 2], fp32)
            nc.sync.dma_start(
                a2[:], wpk_t[OW["a2sd"] : OW["a2sd"] + CLS * 2]
                .rearrange("(p a) -> p a", p=CLS))
            b2w = cp.tile([1, CLS], fp32)
            nc.sync.dma_start(
                b2w[:], wpk_t[OW["b2"] : OW["b2"] + CLS]
                .rearrange("(o a) -> o a", o=1))
            # broadcast biases to 128 partitions via 1-row PE matmul
            ones = cp.tile([1, 128], fp32)
            nc.vector.memset(ones[:], 1.0)
            b1r = cp.tile([128, HC], fp32)
            b2r = cp.tile([128, CLS], fp32)
            # index tables: upload [16, *], replicate into 8 partition groups
            gsrc = cp.tile([128, IW], i16)
            gwd8 = cp.tile([128, IW], u8)
            for k in range(8):
                nc.sync.dma_start(gsrc[16 * k : 16 * (k + 1), :], gsrc_t[:])
                nc.sync.dma_start(gwd8[16 * k : 16 * (k + 1), :], gwdl_t[:])
            # local dst index table: widen u8 dstl + per-block base (b*128)
            gdst = cp.tile([128, IW], i16)
            nc.vector.tensor_copy(out=gdst[:], in_=gwd8[:])
            gbase = cp.tile([128, IW], i16)
            nc.gpsimd.iota(gbase[:], pattern=[[128, BPC], [0, tmax * 8]],
                           base=0, channel_multiplier=0)
            nc.vector.tensor_tensor(out=gdst[:], in0=gdst[:], in1=gbase[:],
                                    op=ALU.add)
            # regroup wrapped u8 -> [128, BPC*tmax] tile-major layout, cast
            dstl8 = cp.tile([128, BPC * tmax], u8)
            for m in range(8):
                nc.sync.dma_start(
                    dstl8[16 * m : 16 * (m + 1), :],
                    gwd8[0:16, :].rearrange("p (s e) -> p s e", e=8)[:, :, m])
            dstlf = cp.tile([128, BPC * tmax], fp32)
            nc.vector.tensor_copy(out=dstlf[:], in_=dstl8[:])
            # zero the one-block pad region of the local gather tables
            zpad = cp.tile([128, HP_W], fp32)
            nc.vector.memset(zpad[:], 0.0)
            nc.sync.dma_start(hpin_t[NPC : NPC + 128, :], zpad[:])
            nc.sync.dma_start(gpin_t[NPC : NPC + 128, :], zpad[:, 0:GP_W])

            # ---- W1T (2 halves) then U = W1 @ Ablk  -> rhs_ext [128,272] ----
            ps = tc.alloc_tile_pool(name="ps_setup", bufs=2, space="PSUM")
            for bw, br, wd in ((b1w, b1r, HC), (b2w, b2r, CLS)):
                bps = ps.tile([128, wd], fp32, tag="bps")
                nc.tensor.matmul(bps[:], lhsT=ones[:], rhs=bw[:],
                                 start=True, stop=True)
                nc.scalar.activation(br[:], bps[:], ACT.Copy)
            w1T = cp.tile([128, 2, 128], fp32)
            rhs_ext = cp.tile([128, HC + 16], fp32)
            nc.scalar.activation(rhs_ext[:, 0:HC], w1[:], ACT.Copy)
            for h in range(2):
                tp = ps.tile([128, 128], fp32, tag="tps")
                nc.tensor.transpose(tp[:], w1[:, h * 128 : (h + 1) * 128], ident[:])
                nc.scalar.activation(w1T[:, h, :], tp[:], ACT.Copy)
            ups = ps.tile([128, 16], fp32, tag="ups")
            for h in range(2):
                nc.tensor.matmul(
                    ups[:], lhsT=w1T[:, h, :], rhs=ablk[:, h, :],
                    start=(h == 0), stop=(h == 1),
                )
            nc.scalar.activation(rhs_ext[:, HC : HC + 16], ups[:], ACT.Copy)

            # ---- rhs2 halves: [W2_half | va_half | vb_half] [128, 2, 42] ----
            rhs2 = cp.tile([128, 2, CLS + 2], fp32)
            for h in range(2):
                vab = ps.tile([128, 2], fp32, tag="vab")
                nc.tensor.matmul(
                    vab[:], lhsT=w2T[:, h * 128 : (h + 1) * 128], rhs=a2[:],
                    start=True, stop=True,
                )
                nc.scalar.activation(rhs2[:, h, 0:CLS], w2h[:, h, :], ACT.Copy)
                nc.scalar.activation(rhs2[:, h, CLS : CLS + 2], vab[:], ACT.Copy)

            ps.release()
            # ---- prologue: h | a_s | a_d for OWN nodes -> hpin, AllGather ----
            ps = tc.alloc_tile_pool(name="ps_pro", bufs=2, space="PSUM")
            with tc.For_i(0, BPC) as ib:
                xb8 = sb.tile([128, 128], f8e4, tag="xb8")
                nc.sync.dma_start(xb8[:], xT_t[:, ts(ib, 128)])
                xb = sb.tile([128, 128], fp32, tag="xb")
                nc.vector.tensor_copy(out=xb[:], in_=xb8[:])
                hps = ps.tile([128, HC + 16], fp32, tag="hps")
                nc.tensor.matmul(
                    hps[:], lhsT=xb[:], rhs=rhs_ext[:],
                    start=True, stop=True,
                )
                hp = sb.tile([128, HP_W], fp32, tag="hp")
                nc.scalar.activation(hp[:, 0 : HC + 16], hps[:], ACT.Copy)
                nc.vector.memset(hp[:, HC + 16 : HP_W], 0.0)
                nc.sync.dma_start(hpin_t[ts(ib, 128), :], hp[:])

            nc.gpsimd.collective_compute(
                "AllGather",
                mybir.AluOpType.bypass,
                replica_groups=[list(range(CORES))],
                ins=[hpin_t[0:NPC, :]],
                outs=[hpk_t[:]],
            )

            cnk_reg = nc.gpsimd.to_reg(CN * 128)
            ps.release()
            ps = tc.alloc_tile_pool(name="ps_l1", bufs=2, space="PSUM")

            # ================= layer 1 edge phase =================
            with tc.For_i(0, BPC) as ib:
                agg = ps.tile([128, HC + 8], fp32, tag="agg")
                gsrc_b = gsrc[:, ts(ib, tmax * 8)]
                gdst_b = gdst[:, ts(ib, tmax * 8)]
                dstl_b = dstlf[:, ts(ib, tmax)]
                for ci in range(nchunks):
                    t0 = ci * CN
                    hg = sb.tile([128, CN, HP_W], fp32, tag="hg")
                    nc.gpsimd.dma_gather(
                        hg[:], hpk_t[:], gsrc_b[:, t0 * 8 : (t0 + CN) * 8],
                        CN * 128, cnk_reg, HP_W, queue_num=0,
                    )
                    adg = sb.tile([128, CN, 64], fp32, tag="adg")
                    nc.gpsimd.dma_gather(
                        adg[:], hpin_t[:, HC : HC + 64],
                        gdst_b[:, t0 * 8 : (t0 + CN) * 8],
                        CN * 128, cnk_reg, 64, elem_step=HP_W, queue_num=1,
                    )
                    w = sb.tile([128, CN, 8], fp32, tag="w")
                    nc.vector.tensor_tensor(
                        out=w[:], in0=hg[:, :, HC : HC + 8],
                        in1=adg[:, :, 8:16], op=ALU.add,
                    )
                    wn = sb.tile([128, CN, 8], fp32, tag="wn")
                    nc.vector.tensor_scalar_mul(wn[:], w[:], NEG)
                    nc.vector.tensor_tensor(out=w[:], in0=w[:], in1=wn[:], op=ALU.max)
                    nc.scalar.activation(w[:], w[:], ACT.Exp)
                    msg = sb.tile([128, CN, HC + 8], fp32, tag="msg")
                    nc.vector.tensor_tensor(
                        out=msg[:, :, 0:HC].rearrange("p c (h y) -> p c h y", y=CH),
                        in0=hg[:, :, 0:HC].rearrange("p c (h y) -> p c h y", y=CH),
                        in1=w[:].unsqueeze(3).to_broadcast([128, CN, 8, CH]),
                        op=ALU.mult,
                    )
                    nc.vector.tensor_copy(out=msg[:, :, HC : HC + 8], in_=w[:])
                    oh = ohp.tile([128, CN, 128], fp32, tag="oh")
                    nc.vector.tensor_tensor(
                        out=oh[:],
                        in0=dstl_b[:, t0 : t0 + CN]
                        .unsqueeze(2).to_broadcast([128, CN, 128]),
                        in1=iotaF[:].unsqueeze(1).to_broadcast([128, CN, 128]),
                        op=ALU.is_equal,
                    )
                    for j in range(CN):
                        nc.tensor.matmul(
                            agg[:], lhsT=oh[:, j, :], rhs=msg[:, j, :],
                            start=(t0 + j == 0), stop=(t0 + j == tmax - 1),
                        )
                # finalize block: y1 = agg/Z + b1; h2 = ELU(y1)
                zc = sb.tile([128, 8], fp32, tag="zc")
                nc.vector.tensor_scalar_max(zc[:], agg[:, HC : HC + 8], 1e-30)
                zr = sb.tile([128, 8], fp32, tag="zr")
                nc.vector.reciprocal(zr[:], zc[:])
                y1 = sb.tile([128, HC], fp32, tag="y1")
                nc.vector.tensor_tensor(
                    out=y1[:].rearrange("p (h y) -> p h y", y=CH),
                    in0=agg[:, 0:HC].rearrange("p (h y) -> p h y", y=CH),
                    in1=zr[:].unsqueeze(2).to_broadcast([128, 8, CH]),
                    op=ALU.mult,
                )
                nc.vector.tensor_tensor(out=y1[:], in0=y1[:], in1=b1r[:], op=ALU.add)
                el = sb.tile([128, HC], fp32, tag="el")
                nc.vector.tensor_scalar_min(el[:], y1[:], 0.0)
                nc.scalar.activation(el[:], el[:], ACT.Exp)
                nc.vector.tensor_scalar_max(y1[:], y1[:], 0.0)
                nc.vector.tensor_tensor(out=y1[:], in0=y1[:], in1=el[:], op=ALU.add)
                nc.vector.tensor_scalar_add(y1[:], y1[:], -1.0)
                nc.sync.dma_start(h2_t[ts(ib, 128), :], y1[:])

            ps.release()
            ps = tc.alloc_tile_pool(name="ps_g", bufs=2, space="PSUM")
            # ================= g table + AllGather =================
            with tc.For_i(0, BPC) as ib:
                h2 = sb.tile([128, HC], fp32, tag="h2")
                nc.sync.dma_start(h2[:], h2_t[ts(ib, 128), :])
                gps = ps.tile([128, CLS + 2], fp32, tag="gps")
                for h in range(2):
                    hTp = ps.tile([128, 128], fp32, tag="hTp")
                    nc.tensor.transpose(
                        hTp[:], h2[:, h * 128 : (h + 1) * 128], ident[:]
                    )
                    hT = sb.tile([128, 128], fp32, tag="hTs")
                    nc.scalar.activation(hT[:], hTp[:], ACT.Copy)
                    nc.tensor.matmul(
                        gps[:], lhsT=hT[:], rhs=rhs2[:, h, :],
                        start=(h == 0), stop=(h == 1),
                    )
                gp = sb.tile([128, GP_W], fp32, tag="gp")
                nc.scalar.activation(gp[:, 0 : CLS + 2], gps[:], ACT.Copy)
                nc.vector.memset(gp[:, CLS + 2 : GP_W], 0.0)
                nc.sync.dma_start(gpin_t[ts(ib, 128), :], gp[:])

            nc.gpsimd.collective_compute(
                "AllGather",
                mybir.AluOpType.bypass,
                replica_groups=[list(range(CORES))],
                ins=[gpin_t[0:NPC, :]],
                outs=[gpk_t[:]],
            )

            ps.release()
            ps = tc.alloc_tile_pool(name="ps_l2", bufs=2, space="PSUM")
            # ================= layer 2 edge phase =================
            with tc.For_i(0, BPC) as ib:
                agg2 = ps.tile([128, CLS + 1], fp32, tag="agg2")
                gsrc_b = gsrc[:, ts(ib, tmax * 8)]
                gdst_b = gdst[:, ts(ib, tmax * 8)]
                dstl_b = dstlf[:, ts(ib, tmax)]
                for ci in range(nchunks):
                    t0 = ci * CN
                    g2 = sb.tile([128, CN, GP_W], fp32, tag="g2")
                    nc.gpsimd.dma_gather(
                        g2[:], gpk_t[:], gsrc_b[:, t0 * 8 : (t0 + CN) * 8],
                        CN * 128, cnk_reg, GP_W, queue_num=0,
                    )
                    ad2 = sb.tile([128, CN, GP_W], fp32, tag="ad2")
                    nc.gpsimd.dma_gather(
                        ad2[:], gpin_t[:], gdst_b[:, t0 * 8 : (t0 + CN) * 8],
                        CN * 128, cnk_reg, GP_W, queue_num=1,
                    )
                    w2 = sb.tile([128, CN, 1], fp32, tag="w2")
                    nc.vector.tensor_tensor(
                        out=w2[:], in0=g2[:, :, CLS : CLS + 1],
                        in1=ad2[:, :, CLS + 1 : CLS + 2], op=ALU.add,
                    )
                    w2n = sb.tile([128, CN, 1], fp32, tag="w2n")
                    nc.vector.tensor_scalar_mul(w2n[:], w2[:], NEG)
                    nc.vector.tensor_tensor(out=w2[:], in0=w2[:], in1=w2n[:], op=ALU.max)
                    nc.scalar.activation(w2[:], w2[:], ACT.Exp)
                    msg2 = sb.tile([128, CN, CLS + 1], fp32, tag="msg2")
                    nc.vector.tensor_tensor(
                        out=msg2[:, :, 0:CLS],
                        in0=g2[:, :, 0:CLS],
                        in1=w2[:].to_broadcast([128, CN, CLS]),
                        op=ALU.mult,
                    )
                    nc.vector.tensor_copy(out=msg2[:, :, CLS : CLS + 1], in_=w2[:])
                    oh = ohp.tile([128, CN, 128], fp32, tag="oh")
                    nc.vector.tensor_tensor(
                        out=oh[:],
                        in0=dstl_b[:, t0 : t0 + CN]
                        .unsqueeze(2).to_broadcast([128, CN, 128]),
                        in1=iotaF[:].unsqueeze(1).to_broadcast([128, CN, 128]),
                        op=ALU.is_equal,
                    )
                    for j in range(CN):
                        nc.tensor.matmul(
                            agg2[:], lhsT=oh[:, j, :], rhs=msg2[:, j, :],
                            start=(t0 + j == 0), stop=(t0 + j == tmax - 1),
                        )
                # finalize: y2 = agg2/Z + b2 -> log_softmax -> out
                z2c = sb.tile([128, 1], fp32, tag="z2c")
                nc.vector.tensor_scalar_max(z2c[:], agg2[:, CLS : CLS + 1], 1e-30)
                z2 = sb.tile([128, 1], fp32, tag="z2")
                nc.vector.reciprocal(z2[:], z2c[:])
                y2 = sb.tile([128, CLS], fp32, tag="y2")
                nc.vector.tensor_scalar(
                    out=y2[:], in0=agg2[:, 0:CLS], scalar1=z2[:, 0:1], scalar2=None,
                    op0=ALU.mult,
                )
                nc.vector.tensor_tensor(out=y2[:], in0=y2[:], in1=b2r[:], op=ALU.add)
                mx = sb.tile([128, 1], fp32, tag="mx")
                nc.vector.reduce_max(mx[:], y2[:], axis=mybir.AxisListType.X)
                nc.vector.tensor_scalar(
                    out=y2[:], in0=y2[:], scalar1=mx[:, 0:1], scalar2=None,
                    op0=ALU.subtract,
                )
                es = sb.tile([128, CLS], fp32, tag="es")
                ssum = sb.tile([128, 1], fp32, tag="ssum")
                nc.scalar.activation(es[:], y2[:], ACT.Exp, accum_out=ssum[:])
                lse = sb.tile([128, 1], fp32, tag="lse")
                nc.scalar.activation(lse[:], ssum[:], ACT.Ln)
                ob = sb.tile([128, CLS], bf16, tag="ob")
                nc.vector.tensor_scalar(
                    out=ob[:], in0=y2[:], scalar1=lse[:, 0:1], scalar2=None,
                    op0=ALU.subtract,
                )
                nc.sync.dma_start(out_t[ts(ib, 128), :], ob[:])
            ps.release()

    nc.finalize()
    return nc


def _host_inputs(inputs, tmax, per_core):
    x = np.asarray(inputs["x"], dtype=np.float32)
    W1 = np.asarray(inputs["W1"], dtype=np.float32)
    a1s = np.asarray(inputs["a1_src"], dtype=np.float32)
    a1d = np.asarray(inputs["a1_dst"], dtype=np.float32)
    b1 = np.asarray(inputs["b1"], dtype=np.float32)
    W2 = np.asarray(inputs["W2"], dtype=np.float32)
    a2s = np.asarray(inputs["a2_src"], dtype=np.float32)
    a2d = np.asarray(inputs["a2_dst"], dtype=np.float32)
    b2 = np.asarray(inputs["b2"], dtype=np.float32)

    xpad = np.zeros((NPAD, F), dtype=np.float32)
    xpad[:N] = x
    ablk = np.zeros((HC, 16), dtype=np.float32)
    for h in range(HEADS):
        ablk[h * CH : (h + 1) * CH, h] = a1s[h]
        ablk[h * CH : (h + 1) * CH, 8 + h] = a1d[h]
    a2sd = np.stack([a2s[0], a2d[0]], axis=1).astype(np.float32)  # [40,2]
    import ml_dtypes

    wblob = np.zeros(WSZ, dtype=np.float32)
    for name, arr in [("W1", W1), ("W2", W2), ("W2T", W2.T), ("Ablk", ablk),
                      ("a2sd", a2sd), ("b1", b1), ("b2", b2)]:
        flat = np.ascontiguousarray(arr).ravel()
        wblob[_W_OFF[name] : _W_OFF[name] + flat.size] = flat

    xpadT16 = xpad.T.astype(ml_dtypes.float8_e4m3)
    maps = []
    for k in range(CORES):
        m = {
            "xT": np.ascontiguousarray(xpadT16[:, k * NPC : (k + 1) * NPC]),
            "wseg": wblob[k * WSEG : (k + 1) * WSEG],
            "gsrc": per_core[k]["gsrc"],
            "gwdl": per_core[k]["gwdl"],
        }
        maps.append(m)
    return maps


def _enable_jax_compile_cache():
    """Persistent XLA executable cache: repeated compiles of the identical
    bass program (run_bass_kernel_spmd re-jits per call) deserialize the
    executable instead of re-running the NEFF pipeline (~0.4s/call)."""
    import os
    import tempfile

    import jax

    cache_dir = os.path.join(tempfile.gettempdir(), "jax_cc_cache")
    try:
        jax.config.update("jax_compilation_cache_dir", cache_dir)
        jax.config.update("jax_persistent_cache_min_compile_time_secs", 0.0)
        jax.config.update("jax_persistent_cache_min_entry_size_bytes", -1)
    except Exception:
        pass


def kernel(**inputs):
    from concourse.bass_utils import run_bass_kernel_spmd

    _enable_jax_compile_cache()
    edge_index = np.asarray(inputs["edge_index"])
    tmax, per_core = _prep_edges(edge_index)

    if tmax not in _cache:
        nc_new = _build_nc(tmax)
        # the module is frozen after finalize(); memoize its serialization
        # on this instance so per-call jit re-lowering skips ~7ms of
        # module_to_json_bytes
        cached_json = nc_new.to_json_bytes()
        nc_new.to_json_bytes = lambda: cached_json
        _cache[tmax] = nc_new
    nc = _cache[tmax]

    in_maps = _host_inputs(inputs, tmax, per_core)
    res = run_bass_kernel_spmd(nc, in_maps, core_ids=list(range(CORES)))
    outs = [np.asarray(res.results[k]["out"]) for k in range(CORES)]
    full = np.concatenate(outs, axis=0)[:N]
    return full.astype(np.float32)
